# revision 34
# baseline (speedup 1.0000x reference)
"""Trainium2 Bass kernel for nn_MoEBlock (top-1 MoE, E=8 experts).

Strategy (8 NeuronCores):
  Launch 1 (gate, token-parallel): each core computes fp32 gating for its
    512-token shard: red = x @ wr.T; logits = red @ normalize(wg).T;
    top-1 score (1/sum(exp(l - max))) and argmax index.
  Host: all-to-all dispatch — gather each expert's tokens (transposed,
    padded to CAP=768) for its owning core.
  Launch 2 (FFN, expert-parallel): core c holds expert c's weights in
    fp32r (TF32-like, full PE rate at N>=256): hT = gelu(W1.T @ xgT + b1),
    outT = (W2.T @ hT + b2) * score. Padded slots carry score=0 so they
    contribute exactly zero. Per-core partial sums reduce to the scalar
    total on-device.
  Host: scatter rows back by token id, add 8 partial sums.

Hardcoded for B=2, T=2048, C=1024, H=4096, E=8 (fixed problem shapes).
"""
import os
import sys

for _p in ("/root/.axon_site/_ro/trn_rl_repo", "/opt/trn_rl_repo"):
    if os.path.isdir(_p) and _p not in sys.path:
        sys.path.append(_p)

import numpy as np

import concourse.bacc as bacc
import concourse.mybir as mybir
import concourse.tile as tile
from concourse.bass_utils import run_bass_kernel_spmd

F32 = mybir.dt.float32
F32R = mybir.dt.float32r
I32 = mybir.dt.int32
U32 = mybir.dt.uint32
AF = mybir.ActivationFunctionType
ALU = mybir.AluOpType

S = 4096          # tokens
C = 1024          # model dim
H = 4096          # ffn dim
E = 8             # experts
RED = 16          # gate reduction dim
NCORES = 8
SHARD = S // NCORES   # tokens per core in the gate kernel
CAP = 768             # max tokens routed to one expert (actual max is 725)

# module-level cache: compiled programs + last exec times
_cache = {}
last_exec_ns = {"gate": None, "ffn": None}


def _trace_flag():
    return bool(int(os.environ.get("MOE_TRACE", "0")))


# --------------------------------------------------------------------------
# Launch 1: gating
# --------------------------------------------------------------------------
def _build_gate():
    nc = bacc.Bacc("TRN2", target_bir_lowering=False, debug=False,
                   num_devices=NCORES)
    KC = C // 128  # 8 K-chunks

    xt_d = nc.dram_tensor("xt", [C, SHARD], F32, kind="ExternalInput").ap()
    wrt_d = nc.dram_tensor("wrt", [C, RED], F32, kind="ExternalInput").ap()
    wg_d = nc.dram_tensor("wg", [E, RED], F32, kind="ExternalInput").ap()
    iden_d = nc.dram_tensor("iden", [E, E], F32, kind="ExternalInput").ap()
    # gout[p, j] = score of token j*128+p; gout[p, 4+j] = its argmax expert
    gout_d = nc.dram_tensor("gout", [128, 8], F32, kind="ExternalOutput").ap()

    with tile.TileContext(nc) as tc:
        with tc.tile_pool(name="sb", bufs=1) as sb, \
             tc.tile_pool(name="sm", bufs=2) as sm, \
             tc.tile_pool(name="ps", bufs=2, space="PSUM") as ps, \
             tc.tile_pool(name="psl", bufs=2, space="PSUM") as psl:
            wrt_sb = sb.tile([128, KC * RED], F32, name="wrt_sb")
            nc.gpsimd.dma_start(
                out=wrt_sb.rearrange("p (k n) -> p k n", k=KC),
                in_=wrt_d.rearrange("(k p) n -> p k n", p=128))
            xt_engs = [nc.sync, nc.sync, nc.scalar, nc.gpsimd,
                       nc.sync, nc.scalar, nc.gpsimd, nc.sync]
            xt_tiles = []
            for k in range(KC):
                t = sb.tile([128, SHARD], F32, name=f"xt{k}", tag=f"xt{k}")
                xt_engs[k].dma_start(out=t, in_=xt_d[k * 128:(k + 1) * 128, :])
                xt_tiles.append(t)
            wg_sb = sb.tile([E, RED], F32, name="wg_sb")
            nc.gpsimd.dma_start(out=wg_sb, in_=wg_d)
            iden_sb = sb.tile([E, E], F32, name="iden_sb")
            nc.gpsimd.dma_start(out=iden_sb, in_=iden_d)

            # --- redT in 4 concurrent column-strips of the PE array ---
            # strip g accumulates k-chunks {g, g+4} into partitions
            # [32g, 32g+16); garbage partitions are zeroed via memset.
            pr = ps.tile([128, SHARD], F32, name="pr")
            nc.vector.memset(pr, 0.0)
            for kk in range(2):
                for g in range(4):
                    k = g + 4 * kk
                    nc.tensor.matmul(pr[32 * g:32 * g + RED, :],
                                     wrt_sb[:, k * RED:(k + 1) * RED],
                                     xt_tiles[k],
                                     start=(kk == 0), stop=(kk == 1),
                                     tile_position=(0, 32 * g))

            # preload the Exp ACT table early (off the critical path)
            dume = sm.tile([1, 1], F32, name="dume")
            nc.scalar.activation(dume, wg_sb[0:1, 0:1], AF.Exp)

            # --- normalize wg rows: wg / max(||wg||, 1e-4) ---
            sq = sm.tile([E, RED], F32, name="sq")
            nc.vector.tensor_tensor(out=sq, in0=wg_sb, in1=wg_sb, op=ALU.mult)
            nrm = sm.tile([E, 1], F32, name="nrm")
            nc.vector.tensor_reduce(out=nrm, in_=sq, axis=mybir.AxisListType.X,
                                    op=ALU.add)
            nc.scalar.sqrt(nrm, nrm)
            nc.vector.tensor_scalar_max(out=nrm, in0=nrm, scalar1=1e-4)
            rcp = sm.tile([E, 1], F32, name="rcp")
            nc.vector.reciprocal(rcp, nrm)
            wgn = sm.tile([E, RED], F32, name="wgn")
            nc.vector.tensor_scalar_mul(out=wgn, in0=wg_sb, scalar1=rcp)

            # --- build wgnt4 [128, E]: wgnt4[32g+r, e] = wgn_n[e, r], zero
            # elsewhere. One K=8 matmul against the identity transposes and
            # replicates into 4 partition strips; the strips let the logits
            # matmul (K=128) sum the 4 partial redT strips for free.
            wgn_rep = sm.tile([E, 128], F32, name="wgn_rep")
            nc.vector.memset(wgn_rep, 0.0)
            for g in range(4):
                nc.vector.tensor_copy(wgn_rep[:, 32 * g:32 * g + RED], wgn)
            pt4 = ps.tile([128, E], F32, name="pt4")
            nc.tensor.matmul(pt4, wgn_rep, iden_sb, start=True, stop=True)
            wgnt4 = sm.tile([128, E], F32, name="wgnt4")
            nc.vector.tensor_copy(wgnt4, pt4)

            red_sb = sb.tile([128, SHARD], F32, name="red_sb")
            nc.vector.tensor_copy(red_sb, pr)

            # --- per 128-token chunk: logits, score, argmax ---
            nchunk = SHARD // 128
            gout_sb = sb.tile([128, 2 * nchunk], F32, name="gout_sb")
            for j in range(nchunk):
                pl = psl.tile([128, E], F32, name="pl")
                nc.tensor.matmul(pl, red_sb[:, j * 128:(j + 1) * 128], wgnt4,
                                 start=True, stop=True)
                lg = sm.tile([128, E], F32, name="lg")
                nc.vector.tensor_copy(lg, pl)
                mx8 = sm.tile([128, 8], F32, name="mx8")
                nc.vector.max(mx8, lg)
                ix8 = sm.tile([128, 8], U32, name="ix8")
                nc.vector.max_index(ix8, mx8, lg)
                nc.vector.tensor_copy(gout_sb[:, nchunk + j:nchunk + j + 1],
                                      ix8[:, 0:1])
                neg = sm.tile([128, 1], F32, name="neg")
                nc.vector.tensor_scalar_mul(out=neg, in0=mx8[:, 0:1],
                                            scalar1=-1.0)
                ex = sm.tile([128, E], F32, name="ex")
                ssum = sm.tile([128, 1], F32, name="ssum")
                nc.scalar.activation(ex, lg, AF.Exp, bias=neg, accum_out=ssum)
                nc.vector.reciprocal(gout_sb[:, j:j + 1], ssum)

            nc.sync.dma_start(out=gout_d, in_=gout_sb)

    nc.compile()
    return nc


# --------------------------------------------------------------------------
# Launch 2: expert FFN (expert pairing + H-split)
#
# Experts are paired (largest token count with smallest). Pair p is handled
# by cores 2p and 2p+1: each core computes HALF of the hidden dim H for BOTH
# experts of the pair, producing partial outputs that the host sums. Slot A
# (cols 0:768) holds the big expert's tokens, slot B (cols 768:1280) the
# small expert's. b2 is pre-halved on the host so both halves add b2/2.
# --------------------------------------------------------------------------
CAPA = 736            # slot A capacity (max expert count is 725); 2x368 MMs
CAPB = 512            # slot B capacity (2nd-smallest..: max small count 491)
A0 = 368              # A-slot matmul split (both halves >=256, bank-aligned
                      # via two separate PSUM tiles)
B0 = 496              # B-slot computed width (max small-expert count is 491;
                      # memory layout stays CAPB wide, compute is trimmed)
TOT = CAPA + CAPB     # 1280 token slots per core
HH = H // 2           # per-core hidden half
# w1 stream blocks (H-chunks each): two small blocks first so the PE can
# start while the bulk DMA is still in flight
W1_BLOCKS = [1, 1] + [2] * 7


KC_CONST = C // 128


def _build_ffn():
    nc = bacc.Bacc("TRN2", target_bir_lowering=False, debug=False,
                   num_devices=NCORES)
    KC = C // 128     # 8
    MCH = HH // 128   # 16 H-chunks in this core's half
    KHALF = MCH // 2  # 8 k-chunks per w2 half-block

    xgt_d = nc.dram_tensor("xgt", [C, TOT], F32R, kind="ExternalInput").ap()
    # weights arrive pre-arranged in SBUF block layout (see _prep_w1/_prep_w2):
    # w1: blocks (W1_BLOCKS H-chunks each) concatenated along the free dim,
    # w2: block (mc,half) -> [128, KHALF*128]
    W1FREE = sum(W1_BLOCKS) * KC_CONST * 128
    w1a_d = nc.dram_tensor("w1a", [128, W1FREE], F32R,
                           kind="ExternalInput").ap()
    w1b_d = nc.dram_tensor("w1b", [128, W1FREE], F32R,
                           kind="ExternalInput").ap()
    w2a_d = nc.dram_tensor("w2a", [KC * 2, 128, (HH // 256) * 128], F32R,
                           kind="ExternalInput").ap()
    w2b_d = nc.dram_tensor("w2b", [KC * 2, 128, (HH // 256) * 128], F32R,
                           kind="ExternalInput").ap()
    b1a_d = nc.dram_tensor("b1a", [HH], F32, kind="ExternalInput").ap()
    b1b_d = nc.dram_tensor("b1b", [HH], F32, kind="ExternalInput").ap()
    b2a_d = nc.dram_tensor("b2a", [C], F32, kind="ExternalInput").ap()
    b2b_d = nc.dram_tensor("b2b", [C], F32, kind="ExternalInput").ap()
    sc_d = nc.dram_tensor("sc", [TOT], F32, kind="ExternalInput").ap()
    out_d = nc.dram_tensor("out", [C, TOT], F32, kind="ExternalOutput").ap()
    tot_d = nc.dram_tensor("tot", [1, 1], F32, kind="ExternalOutput").ap()

    with tile.TileContext(nc) as tc:
        with tc.tile_pool(name="cst", bufs=1) as cst, \
             tc.tile_pool(name="xg", bufs=1) as xg, \
             tc.tile_pool(name="hh", bufs=1) as hhp, \
             tc.tile_pool(name="psA", bufs=2, space="PSUM") as psA, \
             tc.tile_pool(name="psB", bufs=3, space="PSUM") as psB, \
             tc.tile_pool(name="pss", bufs=1, space="PSUM") as pss, \
             tc.tile_pool(name="ot", bufs=3) as ot, \
             tc.tile_pool(name="wp", bufs=6) as wp:
            # early: first (small) w1 blocks for both experts
            blk0len = W1_BLOCKS[0] * KC * 128
            w1blk0a = wp.tile([128, blk0len], F32R, name="wblk", tag="wblk")
            nc.sync.dma_start(out=w1blk0a, in_=w1a_d[:, 0:blk0len])
            w1blk0b = wp.tile([128, blk0len], F32R, name="wblk", tag="wblk")
            nc.scalar.dma_start(out=w1blk0b, in_=w1b_d[:, 0:blk0len])

            xg_engs = [nc.gpsimd, nc.scalar, nc.gpsimd, nc.scalar,
                       nc.gpsimd, nc.scalar, nc.gpsimd, nc.scalar]
            xg_tiles = []
            for k in range(KC):
                t = xg.tile([128, TOT], F32R, name=f"xg{k}", tag=f"xg{k}")
                xg_engs[k].dma_start(out=t, in_=xgt_d[k * 128:(k + 1) * 128, :])
                xg_tiles.append(t)

            b1a_sb = cst.tile([128, MCH], F32, name="b1a_sb")
            nc.gpsimd.dma_start(out=b1a_sb, in_=b1a_d.rearrange("(m p) -> p m", p=128))
            b1b_sb = cst.tile([128, MCH], F32, name="b1b_sb")
            nc.gpsimd.dma_start(out=b1b_sb, in_=b1b_d.rearrange("(m p) -> p m", p=128))
            b2a_sb = cst.tile([128, KC], F32, name="b2a_sb")
            nc.gpsimd.dma_start(out=b2a_sb, in_=b2a_d.rearrange("(m p) -> p m", p=128))
            b2b_sb = cst.tile([128, KC], F32, name="b2b_sb")
            nc.gpsimd.dma_start(out=b2b_sb, in_=b2b_d.rearrange("(m p) -> p m", p=128))
            sc_row = cst.tile([1, TOT], F32, name="sc_row")
            nc.gpsimd.dma_start(out=sc_row, in_=sc_d[None, :])
            ones_row = cst.tile([1, 128], F32, name="ones_row")
            nc.vector.memset(ones_row, 1.0)
            ones_col = cst.tile([128, 1], F32, name="ones_col")
            nc.vector.memset(ones_col, 1.0)

            h_sb = hhp.tile([128, MCH * TOT], F32R, name="h_sb")

            # ---- fc1 ----
            m = 0
            foff = 0
            for mb, blen in enumerate(W1_BLOCKS):
                flen = blen * KC * 128
                if mb == 0:
                    blka, blkb = w1blk0a, w1blk0b
                else:
                    blka = wp.tile([128, flen], F32R, name="wblk", tag="wblk")
                    nc.sync.dma_start(out=blka, in_=w1a_d[:, foff:foff + flen])
                    blkb = wp.tile([128, flen], F32R, name="wblk", tag="wblk")
                    nc.sync.dma_start(out=blkb, in_=w1b_d[:, foff:foff + flen])
                foff += flen
                for mj in range(blen):
                    pA1 = psA.tile([128, A0], F32, name="pA1", tag="pA1")
                    pA2 = psA.tile([128, A0], F32, name="pA2", tag="pA2")
                    pB = psB.tile([128, B0], F32, name="pB")
                    for k in range(KC):
                        off = k * blen * 128 + mj * 128
                        lhsa = blka[:, off:off + 128]
                        lhsb = blkb[:, off:off + 128]
                        nc.tensor.matmul(pA1, lhsa,
                                         xg_tiles[k][:, 0:A0],
                                         start=(k == 0), stop=(k == KC - 1))
                        nc.tensor.matmul(pA2, lhsa,
                                         xg_tiles[k][:, A0:CAPA],
                                         start=(k == 0), stop=(k == KC - 1))
                        nc.tensor.matmul(pB, lhsb,
                                         xg_tiles[k][:, CAPA:CAPA + B0],
                                         start=(k == 0), stop=(k == KC - 1))
                    nc.scalar.activation(h_sb[:, m * TOT:m * TOT + A0], pA1,
                                         AF.Gelu, bias=b1a_sb[:, m:m + 1])
                    nc.scalar.activation(h_sb[:, m * TOT + A0:m * TOT + CAPA], pA2,
                                         AF.Gelu, bias=b1a_sb[:, m:m + 1])
                    nc.scalar.activation(
                        h_sb[:, m * TOT + CAPA:m * TOT + CAPA + B0], pB,
                        AF.Gelu, bias=b1b_sb[:, m:m + 1])
                    m += 1

            # broadcast scores across partitions via K=1 matmul
            # (emitted after fc1 so it doesn't block the PE FIFO at startup)
            sbA1 = psA.tile([128, A0], F32, name="pA1", tag="pA1")
            sbA2 = psA.tile([128, A0], F32, name="pA2", tag="pA2")
            sbB = psB.tile([128, B0], F32, name="pB")
            nc.tensor.matmul(sbA1, ones_row, sc_row[:, 0:A0],
                             start=True, stop=True)
            nc.tensor.matmul(sbA2, ones_row, sc_row[:, A0:CAPA],
                             start=True, stop=True)
            nc.tensor.matmul(sbB, ones_row, sc_row[:, CAPA:CAPA + B0],
                             start=True, stop=True)
            scb = cst.tile([128, TOT], F32, name="scb")
            nc.vector.tensor_copy(scb[:, 0:A0], sbA1)
            nc.vector.tensor_copy(scb[:, A0:CAPA], sbA2)
            nc.vector.tensor_copy(scb[:, CAPA:CAPA + B0], sbB)

            # ---- fc2 ----
            part_sb = cst.tile([128, 3 * KC], F32, name="part_sb")
            for mc in range(KC):
                pA1 = psA.tile([128, A0], F32, name="pA1", tag="pA1")
                pA2 = psA.tile([128, A0], F32, name="pA2", tag="pA2")
                pB = psB.tile([128, B0], F32, name="pB")
                for half in range(2):
                    blka = wp.tile([128, KHALF * 128], F32R, name="wblk")
                    nc.sync.dma_start(out=blka, in_=w2a_d[mc * 2 + half])
                    blkb = wp.tile([128, KHALF * 128], F32R, name="wblk")
                    nc.sync.dma_start(out=blkb, in_=w2b_d[mc * 2 + half])
                    for kk in range(KHALF):
                        k = half * KHALF + kk
                        lhsa = blka[:, kk * 128:(kk + 1) * 128]
                        lhsb = blkb[:, kk * 128:(kk + 1) * 128]
                        nc.tensor.matmul(pA1, lhsa,
                                         h_sb[:, k * TOT:k * TOT + A0],
                                         start=(k == 0), stop=(k == MCH - 1))
                        nc.tensor.matmul(pA2, lhsa,
                                         h_sb[:, k * TOT + A0:k * TOT + CAPA],
                                         start=(k == 0), stop=(k == MCH - 1))
                        nc.tensor.matmul(pB, lhsb,
                                         h_sb[:, k * TOT + CAPA:k * TOT + CAPA + B0],
                                         start=(k == 0), stop=(k == MCH - 1))
                o = ot.tile([128, TOT], F32, name="o")
                nc.vector.scalar_tensor_tensor(
                    out=o[:, 0:A0], in0=pA1, scalar=b2a_sb[:, mc:mc + 1],
                    in1=scb[:, 0:A0], op0=ALU.add, op1=ALU.mult,
                    accum_out=part_sb[:, 3 * mc:3 * mc + 1])
                nc.vector.scalar_tensor_tensor(
                    out=o[:, A0:CAPA], in0=pA2, scalar=b2a_sb[:, mc:mc + 1],
                    in1=scb[:, A0:CAPA], op0=ALU.add, op1=ALU.mult,
                    accum_out=part_sb[:, 3 * mc + 1:3 * mc + 2])
                nc.vector.scalar_tensor_tensor(
                    out=o[:, CAPA:CAPA + B0], in0=pB,
                    scalar=b2b_sb[:, mc:mc + 1],
                    in1=scb[:, CAPA:CAPA + B0], op0=ALU.add, op1=ALU.mult,
                    accum_out=part_sb[:, 3 * mc + 2:3 * mc + 3])
                nc.scalar.dma_start(out=out_d[mc * 128:(mc + 1) * 128, :], in_=o)

            # ---- total = sum of partials ----
            rsum = cst.tile([128, 1], F32, name="rsum")
            nc.vector.tensor_reduce(out=rsum, in_=part_sb,
                                    axis=mybir.AxisListType.X, op=ALU.add)
            ptot = pss.tile([1, 1], F32, name="ptot")
            nc.tensor.matmul(ptot, ones_col, rsum, start=True, stop=True)
            tot_sb = cst.tile([1, 1], F32, name="tot_sb")
            nc.vector.tensor_copy(tot_sb, ptot)
            nc.sync.dma_start(out=tot_d, in_=tot_sb)

    nc.compile()
    return nc


def _get(name, builder):
    if name not in _cache:
        _cache[name] = builder()
    return _cache[name]


def _run(nc, in_maps, tag):
    if _trace_flag():
        try:
            res = run_bass_kernel_spmd(nc, in_maps,
                                       core_ids=list(range(NCORES)), trace=True)
            last_exec_ns[tag] = res.exec_time_ns
            return res.results
        except Exception as e:
            print(f"trace run failed ({e}); falling back to untraced",
                  file=sys.stderr)
    res = run_bass_kernel_spmd(nc, in_maps, core_ids=list(range(NCORES)))
    return res.results


# --------------------------------------------------------------------------
# Host orchestration
# --------------------------------------------------------------------------
def kernel(x, wr, wg, w1, b1, w2, b2):
    x = np.ascontiguousarray(np.asarray(x, dtype=np.float32))
    wr = np.ascontiguousarray(np.asarray(wr, dtype=np.float32))
    wg = np.ascontiguousarray(np.asarray(wg, dtype=np.float32))
    w1 = np.ascontiguousarray(np.asarray(w1, dtype=np.float32))
    b1 = np.ascontiguousarray(np.asarray(b1, dtype=np.float32))
    w2 = np.ascontiguousarray(np.asarray(w2, dtype=np.float32))
    b2 = np.ascontiguousarray(np.asarray(b2, dtype=np.float32))

    B, T, _ = x.shape
    xf = x.reshape(S, C)
    xT = np.ascontiguousarray(xf.T)            # [C, S]
    wrt = np.ascontiguousarray(wr.T)           # [C, RED]
    iden = np.eye(E, dtype=np.float32)

    # ---- launch 1: gating (token-parallel shards) ----
    gate_nc = _get("gate", _build_gate)
    in_maps = [{
        "xt": np.ascontiguousarray(xT[:, c * SHARD:(c + 1) * SHARD]),
        "wrt": wrt, "wg": wg, "iden": iden,
    } for c in range(NCORES)]
    gres = _run(gate_nc, in_maps, "gate")
    nch = SHARD // 128
    score = np.concatenate(
        [gres[c]["gout"][:, :nch].T.ravel() for c in range(NCORES)])
    idx = np.concatenate(
        [gres[c]["gout"][:, nch:].T.ravel() for c in range(NCORES)]
    ).astype(np.int64)

    def _prep_w1(w):          # [C, HH] -> [128, sum(blocks)*KC*128]
        kc, mch = C // 128, HH // 128
        wr4 = w.reshape(kc, 128, mch, 128)
        parts = []
        m0 = 0
        for blen in W1_BLOCKS:
            blk = wr4[:, :, m0:m0 + blen, :]         # [kc,128,blen,128]
            parts.append(blk.transpose(1, 0, 2, 3).reshape(128, kc * blen * 128))
            m0 += blen
        return np.ascontiguousarray(np.concatenate(parts, axis=1))

    def _prep_w2(w):          # [HH, C] -> [KC*2, 128, KHALF*128]
        kc, khalf = C // 128, (HH // 128) // 2
        return np.ascontiguousarray(
            w.reshape(2, khalf, 128, kc, 128).transpose(3, 0, 2, 1, 4)
             .reshape(2 * kc, 128, khalf * 128))

    # ---- host all-to-all dispatch: pair experts, split H across 2 cores ----
    counts = np.bincount(idx, minlength=E)
    order = np.argsort(-counts, kind="stable")
    pairs = [(int(order[i]), int(order[E - 1 - i])) for i in range(E // 2)]
    feasible = all(counts[ea] <= CAPA and counts[eb] <= B0
                   for ea, eb in pairs)
    if not feasible:
        # Safety net for out-of-distribution routing (cannot happen for the
        # fixed problem seed): exact dense-per-token fallback on host.
        out = np.empty((S, C), dtype=np.float32)
        for e in range(E):
            tok = np.nonzero(idx == e)[0]
            if tok.size == 0:
                continue
            hmid = xf[tok] @ w1[e] + b1[e]
            from scipy.special import erf
            hmid = 0.5 * hmid * (1.0 + erf(hmid / np.sqrt(2.0)))
            out[tok] = ((hmid @ w2[e] + b2[e])
                        * score[tok][:, None]).astype(np.float32)
        return out.reshape(B, T, C), np.float32(out.sum(dtype=np.float32))

    token_lists = []   # per pair: (tok_a, tok_b)
    ffn_maps = []
    for p, (ea, eb) in enumerate(pairs):
        tok_a = np.nonzero(idx == ea)[0]
        tok_b = np.nonzero(idx == eb)[0]
        token_lists.append((tok_a, tok_b))
        xgt = np.zeros((C, TOT), dtype=np.float32)
        xgt[:, :tok_a.size] = xT[:, tok_a]
        xgt[:, CAPA:CAPA + tok_b.size] = xT[:, tok_b]
        sc = np.zeros(TOT, dtype=np.float32)
        sc[:tok_a.size] = score[tok_a]
        sc[CAPA:CAPA + tok_b.size] = score[tok_b]
        b2a = (b2[ea] * 0.5).astype(np.float32)
        b2b = (b2[eb] * 0.5).astype(np.float32)
        for half in range(2):
            hs = slice(half * HH, (half + 1) * HH)
            ffn_maps.append({
                "xgt": xgt, "sc": sc,
                "w1a": _prep_w1(w1[ea][:, hs]),
                "w1b": _prep_w1(w1[eb][:, hs]),
                "w2a": _prep_w2(w2[ea][hs, :]),
                "w2b": _prep_w2(w2[eb][hs, :]),
                "b1a": np.ascontiguousarray(b1[ea][hs]),
                "b1b": np.ascontiguousarray(b1[eb][hs]),
                "b2a": b2a, "b2b": b2b,
            })

    # ---- launch 2: expert FFN ----
    ffn_nc = _get("ffn", _build_ffn)
    fres = _run(ffn_nc, ffn_maps, "ffn")

    # ---- combine: host sums the two H-half partials, scatters by token ----
    out = np.empty((S, C), dtype=np.float32)
    total = np.float32(0.0)
    for p, (ea, eb) in enumerate(pairs):
        tok_a, tok_b = token_lists[p]
        o0 = fres[2 * p]["out"]
        o1 = fres[2 * p + 1]["out"]
        out[tok_a] = (o0[:, :tok_a.size] + o1[:, :tok_a.size]).T
        out[tok_b] = (o0[:, CAPA:CAPA + tok_b.size]
                      + o1[:, CAPA:CAPA + tok_b.size]).T
        total = np.float32(total + fres[2 * p]["tot"][0, 0]
                           + fres[2 * p + 1]["tot"][0, 0])
    return out.reshape(B, T, C), total


# revision 35
# speedup vs baseline: 1.0045x; 1.0045x over previous
"""Trainium2 Bass kernel for nn_MoEBlock (top-1 MoE, E=8 experts).

Strategy (8 NeuronCores):
  Launch 1 (gate, token-parallel): each core computes fp32 gating for its
    512-token shard: red = x @ wr.T; logits = red @ normalize(wg).T;
    top-1 score (1/sum(exp(l - max))) and argmax index.
  Host: all-to-all dispatch — gather each expert's tokens (transposed,
    padded to CAP=768) for its owning core.
  Launch 2 (FFN, expert-parallel): core c holds expert c's weights in
    fp32r (TF32-like, full PE rate at N>=256): hT = gelu(W1.T @ xgT + b1),
    outT = (W2.T @ hT + b2) * score. Padded slots carry score=0 so they
    contribute exactly zero. Per-core partial sums reduce to the scalar
    total on-device.
  Host: scatter rows back by token id, add 8 partial sums.

Hardcoded for B=2, T=2048, C=1024, H=4096, E=8 (fixed problem shapes).
"""
import os
import sys

for _p in ("/root/.axon_site/_ro/trn_rl_repo", "/opt/trn_rl_repo"):
    if os.path.isdir(_p) and _p not in sys.path:
        sys.path.append(_p)

import numpy as np

import concourse.bacc as bacc
import concourse.mybir as mybir
import concourse.tile as tile
from concourse.bass_utils import run_bass_kernel_spmd

F32 = mybir.dt.float32
F32R = mybir.dt.float32r
I32 = mybir.dt.int32
U32 = mybir.dt.uint32
AF = mybir.ActivationFunctionType
ALU = mybir.AluOpType

S = 4096          # tokens
C = 1024          # model dim
H = 4096          # ffn dim
E = 8             # experts
RED = 16          # gate reduction dim
NCORES = 8
SHARD = S // NCORES   # tokens per core in the gate kernel
CAP = 768             # max tokens routed to one expert (actual max is 725)

# module-level cache: compiled programs + last exec times
_cache = {}
last_exec_ns = {"gate": None, "ffn": None}


def _trace_flag():
    return bool(int(os.environ.get("MOE_TRACE", "0")))


# --------------------------------------------------------------------------
# Launch 1: gating
# --------------------------------------------------------------------------
def _build_gate():
    nc = bacc.Bacc("TRN2", target_bir_lowering=False, debug=False,
                   num_devices=NCORES)
    KC = C // 128  # 8 K-chunks

    xt_d = nc.dram_tensor("xt", [C, SHARD], F32, kind="ExternalInput").ap()
    wrt_d = nc.dram_tensor("wrt", [C, RED], F32, kind="ExternalInput").ap()
    wg_d = nc.dram_tensor("wg", [E, RED], F32, kind="ExternalInput").ap()
    iden_d = nc.dram_tensor("iden", [E, E], F32, kind="ExternalInput").ap()
    # gout[p, j] = score of token j*128+p; gout[p, 4+j] = its argmax expert
    gout_d = nc.dram_tensor("gout", [128, 8], F32, kind="ExternalOutput").ap()

    with tile.TileContext(nc) as tc:
        with tc.tile_pool(name="sb", bufs=1) as sb, \
             tc.tile_pool(name="sm", bufs=2) as sm, \
             tc.tile_pool(name="ps", bufs=2, space="PSUM") as ps, \
             tc.tile_pool(name="psl", bufs=2, space="PSUM") as psl:
            wrt_sb = sb.tile([128, KC * RED], F32, name="wrt_sb")
            nc.gpsimd.dma_start(
                out=wrt_sb.rearrange("p (k n) -> p k n", k=KC),
                in_=wrt_d.rearrange("(k p) n -> p k n", p=128))
            xt_engs = [nc.sync, nc.sync, nc.scalar, nc.gpsimd,
                       nc.sync, nc.scalar, nc.gpsimd, nc.sync]
            xt_tiles = []
            for k in range(KC):
                t = sb.tile([128, SHARD], F32, name=f"xt{k}", tag=f"xt{k}")
                xt_engs[k].dma_start(out=t, in_=xt_d[k * 128:(k + 1) * 128, :])
                xt_tiles.append(t)
            wg_sb = sb.tile([E, RED], F32, name="wg_sb")
            nc.gpsimd.dma_start(out=wg_sb, in_=wg_d)
            iden_sb = sb.tile([E, E], F32, name="iden_sb")
            nc.gpsimd.dma_start(out=iden_sb, in_=iden_d)

            # --- redT in 4 concurrent column-strips of the PE array ---
            # strip g accumulates k-chunks {g, g+4} into partitions
            # [32g, 32g+16); garbage partitions are zeroed via memset.
            pr = ps.tile([128, SHARD], F32, name="pr")
            nc.vector.memset(pr, 0.0)
            for kk in range(2):
                for g in range(4):
                    k = g + 4 * kk
                    nc.tensor.matmul(pr[32 * g:32 * g + RED, :],
                                     wrt_sb[:, k * RED:(k + 1) * RED],
                                     xt_tiles[k],
                                     start=(kk == 0), stop=(kk == 1),
                                     tile_position=(0, 32 * g))

            # preload the Exp ACT table early (off the critical path)
            dume = sm.tile([1, 1], F32, name="dume")
            nc.scalar.activation(dume, wg_sb[0:1, 0:1], AF.Exp)

            # --- normalize wg rows: wg / max(||wg||, 1e-4) ---
            sq = sm.tile([E, RED], F32, name="sq")
            nc.vector.tensor_tensor(out=sq, in0=wg_sb, in1=wg_sb, op=ALU.mult)
            nrm = sm.tile([E, 1], F32, name="nrm")
            nc.vector.tensor_reduce(out=nrm, in_=sq, axis=mybir.AxisListType.X,
                                    op=ALU.add)
            nc.scalar.sqrt(nrm, nrm)
            nc.vector.tensor_scalar_max(out=nrm, in0=nrm, scalar1=1e-4)
            rcp = sm.tile([E, 1], F32, name="rcp")
            nc.vector.reciprocal(rcp, nrm)
            wgn = sm.tile([E, RED], F32, name="wgn")
            nc.vector.tensor_scalar_mul(out=wgn, in0=wg_sb, scalar1=rcp)

            # --- build wgnt4 [128, E]: wgnt4[32g+r, e] = wgn_n[e, r], zero
            # elsewhere. One K=8 matmul against the identity transposes and
            # replicates into 4 partition strips; the strips let the logits
            # matmul (K=128) sum the 4 partial redT strips for free.
            wgn_rep = sm.tile([E, 128], F32, name="wgn_rep")
            nc.vector.memset(wgn_rep, 0.0)
            for g in range(4):
                nc.vector.tensor_copy(wgn_rep[:, 32 * g:32 * g + RED], wgn)
            pt4 = ps.tile([128, E], F32, name="pt4")
            nc.tensor.matmul(pt4, wgn_rep, iden_sb, start=True, stop=True)
            wgnt4 = sm.tile([128, E], F32, name="wgnt4")
            nc.vector.tensor_copy(wgnt4, pt4)

            red_sb = sb.tile([128, SHARD], F32, name="red_sb")
            nc.vector.tensor_copy(red_sb, pr)

            # --- per 128-token chunk: logits, score, argmax ---
            nchunk = SHARD // 128
            gout_sb = sb.tile([128, 2 * nchunk], F32, name="gout_sb")
            for j in range(nchunk):
                pl = psl.tile([128, E], F32, name="pl")
                nc.tensor.matmul(pl, red_sb[:, j * 128:(j + 1) * 128], wgnt4,
                                 start=True, stop=True)
                lg = sm.tile([128, E], F32, name="lg")
                nc.vector.tensor_copy(lg, pl)
                mx8 = sm.tile([128, 8], F32, name="mx8")
                nc.vector.max(mx8, lg)
                ix8 = sm.tile([128, 8], U32, name="ix8")
                nc.vector.max_index(ix8, mx8, lg)
                nc.vector.tensor_copy(gout_sb[:, nchunk + j:nchunk + j + 1],
                                      ix8[:, 0:1])
                neg = sm.tile([128, 1], F32, name="neg")
                nc.vector.tensor_scalar_mul(out=neg, in0=mx8[:, 0:1],
                                            scalar1=-1.0)
                ex = sm.tile([128, E], F32, name="ex")
                ssum = sm.tile([128, 1], F32, name="ssum")
                nc.scalar.activation(ex, lg, AF.Exp, bias=neg, accum_out=ssum)
                nc.vector.reciprocal(gout_sb[:, j:j + 1], ssum)

            nc.sync.dma_start(out=gout_d, in_=gout_sb)

    nc.compile()
    return nc


# --------------------------------------------------------------------------
# Launch 2: expert FFN (expert pairing + H-split)
#
# Experts are paired (largest token count with smallest). Pair p is handled
# by cores 2p and 2p+1: each core computes HALF of the hidden dim H for BOTH
# experts of the pair, producing partial outputs that the host sums. Slot A
# (cols 0:768) holds the big expert's tokens, slot B (cols 768:1280) the
# small expert's. b2 is pre-halved on the host so both halves add b2/2.
# --------------------------------------------------------------------------
CAPA = 736            # slot A capacity (max expert count is 725); 2x368 MMs
CAPB = 512            # slot B capacity (2nd-smallest..: max small count 491)
A0 = 368              # A-slot matmul split (both halves >=256, bank-aligned
                      # via two separate PSUM tiles)
B0 = 496              # B-slot computed width (max small-expert count is 491;
                      # memory layout stays CAPB wide, compute is trimmed)
TOT = CAPA + CAPB     # 1280 token slots per core
HH = H // 2           # per-core hidden half
# w1 stream blocks (H-chunks each): two small blocks first so the PE can
# start while the bulk DMA is still in flight
W1_BLOCKS = [1, 1] + [2] * 7


KC_CONST = C // 128


def _build_ffn():
    nc = bacc.Bacc("TRN2", target_bir_lowering=False, debug=False,
                   num_devices=NCORES)
    KC = C // 128     # 8
    MCH = HH // 128   # 16 H-chunks in this core's half
    KHALF = MCH // 2  # 8 k-chunks per w2 half-block

    xgt_d = nc.dram_tensor("xgt", [C, TOT], F32R, kind="ExternalInput").ap()
    # weights arrive pre-arranged in SBUF block layout (see _prep_w1/_prep_w2):
    # w1: blocks (W1_BLOCKS H-chunks each) concatenated along the free dim,
    # w2: block (mc,half) -> [128, KHALF*128]
    W1FREE = sum(W1_BLOCKS) * KC_CONST * 128
    w1a_d = nc.dram_tensor("w1a", [128, W1FREE], F32R,
                           kind="ExternalInput").ap()
    w1b_d = nc.dram_tensor("w1b", [128, W1FREE], F32R,
                           kind="ExternalInput").ap()
    w2a_d = nc.dram_tensor("w2a", [KC * 2, 128, (HH // 256) * 128], F32R,
                           kind="ExternalInput").ap()
    w2b_d = nc.dram_tensor("w2b", [KC * 2, 128, (HH // 256) * 128], F32R,
                           kind="ExternalInput").ap()
    b1a_d = nc.dram_tensor("b1a", [HH], F32, kind="ExternalInput").ap()
    b1b_d = nc.dram_tensor("b1b", [HH], F32, kind="ExternalInput").ap()
    b2a_d = nc.dram_tensor("b2a", [C], F32, kind="ExternalInput").ap()
    b2b_d = nc.dram_tensor("b2b", [C], F32, kind="ExternalInput").ap()
    sc_d = nc.dram_tensor("sc", [TOT], F32, kind="ExternalInput").ap()
    out_d = nc.dram_tensor("out", [C, TOT], F32, kind="ExternalOutput").ap()
    tot_d = nc.dram_tensor("tot", [1, 1], F32, kind="ExternalOutput").ap()

    with tile.TileContext(nc) as tc:
        with tc.tile_pool(name="cst", bufs=1) as cst, \
             tc.tile_pool(name="xg", bufs=1) as xg, \
             tc.tile_pool(name="hh", bufs=1) as hhp, \
             tc.tile_pool(name="psA", bufs=2, space="PSUM") as psA, \
             tc.tile_pool(name="psB", bufs=2, space="PSUM") as psB, \
             tc.tile_pool(name="pss", bufs=1, space="PSUM") as pss, \
             tc.tile_pool(name="ot", bufs=2) as ot, \
             tc.tile_pool(name="wp", bufs=6) as wp:
            # early: first (small) w1 blocks for both experts
            blk0len = W1_BLOCKS[0] * KC * 128
            w1blk0a = wp.tile([128, blk0len], F32R, name="wblk", tag="wblk")
            nc.sync.dma_start(out=w1blk0a, in_=w1a_d[:, 0:blk0len])
            w1blk0b = wp.tile([128, blk0len], F32R, name="wblk", tag="wblk")
            nc.scalar.dma_start(out=w1blk0b, in_=w1b_d[:, 0:blk0len])

            xg_engs = [nc.gpsimd, nc.scalar, nc.gpsimd, nc.scalar,
                       nc.gpsimd, nc.scalar, nc.gpsimd, nc.scalar]
            xg_tiles = []
            for k in range(KC):
                t = xg.tile([128, TOT], F32R, name=f"xg{k}", tag=f"xg{k}")
                xg_engs[k].dma_start(out=t, in_=xgt_d[k * 128:(k + 1) * 128, :])
                xg_tiles.append(t)

            b1a_sb = cst.tile([128, MCH], F32, name="b1a_sb")
            nc.gpsimd.dma_start(out=b1a_sb, in_=b1a_d.rearrange("(m p) -> p m", p=128))
            b1b_sb = cst.tile([128, MCH], F32, name="b1b_sb")
            nc.gpsimd.dma_start(out=b1b_sb, in_=b1b_d.rearrange("(m p) -> p m", p=128))
            b2a_sb = cst.tile([128, KC], F32, name="b2a_sb")
            nc.gpsimd.dma_start(out=b2a_sb, in_=b2a_d.rearrange("(m p) -> p m", p=128))
            b2b_sb = cst.tile([128, KC], F32, name="b2b_sb")
            nc.gpsimd.dma_start(out=b2b_sb, in_=b2b_d.rearrange("(m p) -> p m", p=128))
            sc_row = cst.tile([1, TOT], F32, name="sc_row")
            nc.gpsimd.dma_start(out=sc_row, in_=sc_d[None, :])
            ones_row = cst.tile([1, 128], F32, name="ones_row")
            nc.vector.memset(ones_row, 1.0)
            ones_col = cst.tile([128, 1], F32, name="ones_col")
            nc.vector.memset(ones_col, 1.0)

            h_sb = hhp.tile([128, MCH * TOT], F32R, name="h_sb")

            # ---- fc1 ----
            m = 0
            foff = 0
            for mb, blen in enumerate(W1_BLOCKS):
                flen = blen * KC * 128
                if mb == 0:
                    blka, blkb = w1blk0a, w1blk0b
                else:
                    blka = wp.tile([128, flen], F32R, name="wblk", tag="wblk")
                    nc.sync.dma_start(out=blka, in_=w1a_d[:, foff:foff + flen])
                    blkb = wp.tile([128, flen], F32R, name="wblk", tag="wblk")
                    nc.sync.dma_start(out=blkb, in_=w1b_d[:, foff:foff + flen])
                foff += flen
                for mj in range(blen):
                    pA1 = psA.tile([128, A0], F32, name="pA1", tag="pA1")
                    pA2 = psA.tile([128, A0], F32, name="pA2", tag="pA2")
                    pB = psB.tile([128, B0], F32, name="pB")
                    for k in range(KC):
                        off = k * blen * 128 + mj * 128
                        lhsa = blka[:, off:off + 128]
                        lhsb = blkb[:, off:off + 128]
                        nc.tensor.matmul(pA1, lhsa,
                                         xg_tiles[k][:, 0:A0],
                                         start=(k == 0), stop=(k == KC - 1))
                        nc.tensor.matmul(pA2, lhsa,
                                         xg_tiles[k][:, A0:CAPA],
                                         start=(k == 0), stop=(k == KC - 1))
                        nc.tensor.matmul(pB, lhsb,
                                         xg_tiles[k][:, CAPA:CAPA + B0],
                                         start=(k == 0), stop=(k == KC - 1))
                    nc.scalar.activation(h_sb[:, m * TOT:m * TOT + A0], pA1,
                                         AF.Gelu, bias=b1a_sb[:, m:m + 1])
                    nc.scalar.activation(h_sb[:, m * TOT + A0:m * TOT + CAPA], pA2,
                                         AF.Gelu, bias=b1a_sb[:, m:m + 1])
                    nc.scalar.activation(
                        h_sb[:, m * TOT + CAPA:m * TOT + CAPA + B0], pB,
                        AF.Gelu, bias=b1b_sb[:, m:m + 1])
                    m += 1

            # broadcast scores across partitions via K=1 matmul
            # (emitted after fc1 so it doesn't block the PE FIFO at startup)
            sbA1 = psA.tile([128, A0], F32, name="pA1", tag="pA1")
            sbA2 = psA.tile([128, A0], F32, name="pA2", tag="pA2")
            sbB = psB.tile([128, B0], F32, name="pB")
            nc.tensor.matmul(sbA1, ones_row, sc_row[:, 0:A0],
                             start=True, stop=True)
            nc.tensor.matmul(sbA2, ones_row, sc_row[:, A0:CAPA],
                             start=True, stop=True)
            nc.tensor.matmul(sbB, ones_row, sc_row[:, CAPA:CAPA + B0],
                             start=True, stop=True)
            scb = cst.tile([128, TOT], F32, name="scb")
            nc.vector.tensor_copy(scb[:, 0:A0], sbA1)
            nc.vector.tensor_copy(scb[:, A0:CAPA], sbA2)
            nc.vector.tensor_copy(scb[:, CAPA:CAPA + B0], sbB)

            # ---- fc2 ----
            part_sb = cst.tile([128, 3 * KC], F32, name="part_sb")
            for mc in range(KC):
                pA1 = psA.tile([128, A0], F32, name="pA1", tag="pA1")
                pA2 = psA.tile([128, A0], F32, name="pA2", tag="pA2")
                pB = psB.tile([128, B0], F32, name="pB")
                for half in range(2):
                    blka = wp.tile([128, KHALF * 128], F32R, name="wblk")
                    nc.sync.dma_start(out=blka, in_=w2a_d[mc * 2 + half])
                    blkb = wp.tile([128, KHALF * 128], F32R, name="wblk")
                    nc.sync.dma_start(out=blkb, in_=w2b_d[mc * 2 + half])
                    for kk in range(KHALF):
                        k = half * KHALF + kk
                        lhsa = blka[:, kk * 128:(kk + 1) * 128]
                        lhsb = blkb[:, kk * 128:(kk + 1) * 128]
                        nc.tensor.matmul(pA1, lhsa,
                                         h_sb[:, k * TOT:k * TOT + A0],
                                         start=(k == 0), stop=(k == MCH - 1))
                        nc.tensor.matmul(pA2, lhsa,
                                         h_sb[:, k * TOT + A0:k * TOT + CAPA],
                                         start=(k == 0), stop=(k == MCH - 1))
                        nc.tensor.matmul(pB, lhsb,
                                         h_sb[:, k * TOT + CAPA:k * TOT + CAPA + B0],
                                         start=(k == 0), stop=(k == MCH - 1))
                o = ot.tile([128, TOT], F32, name="o")
                nc.vector.scalar_tensor_tensor(
                    out=o[:, 0:A0], in0=pA1, scalar=b2a_sb[:, mc:mc + 1],
                    in1=scb[:, 0:A0], op0=ALU.add, op1=ALU.mult,
                    accum_out=part_sb[:, 3 * mc:3 * mc + 1])
                nc.vector.scalar_tensor_tensor(
                    out=o[:, A0:CAPA], in0=pA2, scalar=b2a_sb[:, mc:mc + 1],
                    in1=scb[:, A0:CAPA], op0=ALU.add, op1=ALU.mult,
                    accum_out=part_sb[:, 3 * mc + 1:3 * mc + 2])
                nc.vector.scalar_tensor_tensor(
                    out=o[:, CAPA:CAPA + B0], in0=pB,
                    scalar=b2b_sb[:, mc:mc + 1],
                    in1=scb[:, CAPA:CAPA + B0], op0=ALU.add, op1=ALU.mult,
                    accum_out=part_sb[:, 3 * mc + 2:3 * mc + 3])
                nc.scalar.dma_start(out=out_d[mc * 128:(mc + 1) * 128, :], in_=o)

            # ---- total = sum of partials ----
            rsum = cst.tile([128, 1], F32, name="rsum")
            nc.vector.tensor_reduce(out=rsum, in_=part_sb,
                                    axis=mybir.AxisListType.X, op=ALU.add)
            ptot = pss.tile([1, 1], F32, name="ptot")
            nc.tensor.matmul(ptot, ones_col, rsum, start=True, stop=True)
            tot_sb = cst.tile([1, 1], F32, name="tot_sb")
            nc.vector.tensor_copy(tot_sb, ptot)
            nc.sync.dma_start(out=tot_d, in_=tot_sb)

    nc.compile()
    return nc


def _get(name, builder):
    if name not in _cache:
        _cache[name] = builder()
    return _cache[name]


def _run(nc, in_maps, tag):
    if _trace_flag():
        try:
            res = run_bass_kernel_spmd(nc, in_maps,
                                       core_ids=list(range(NCORES)), trace=True)
            last_exec_ns[tag] = res.exec_time_ns
            return res.results
        except Exception as e:
            print(f"trace run failed ({e}); falling back to untraced",
                  file=sys.stderr)
    res = run_bass_kernel_spmd(nc, in_maps, core_ids=list(range(NCORES)))
    return res.results


# --------------------------------------------------------------------------
# Host orchestration
# --------------------------------------------------------------------------
def kernel(x, wr, wg, w1, b1, w2, b2):
    x = np.ascontiguousarray(np.asarray(x, dtype=np.float32))
    wr = np.ascontiguousarray(np.asarray(wr, dtype=np.float32))
    wg = np.ascontiguousarray(np.asarray(wg, dtype=np.float32))
    w1 = np.ascontiguousarray(np.asarray(w1, dtype=np.float32))
    b1 = np.ascontiguousarray(np.asarray(b1, dtype=np.float32))
    w2 = np.ascontiguousarray(np.asarray(w2, dtype=np.float32))
    b2 = np.ascontiguousarray(np.asarray(b2, dtype=np.float32))

    B, T, _ = x.shape
    xf = x.reshape(S, C)
    xT = np.ascontiguousarray(xf.T)            # [C, S]
    wrt = np.ascontiguousarray(wr.T)           # [C, RED]
    iden = np.eye(E, dtype=np.float32)

    # ---- launch 1: gating (token-parallel shards) ----
    gate_nc = _get("gate", _build_gate)
    in_maps = [{
        "xt": np.ascontiguousarray(xT[:, c * SHARD:(c + 1) * SHARD]),
        "wrt": wrt, "wg": wg, "iden": iden,
    } for c in range(NCORES)]
    gres = _run(gate_nc, in_maps, "gate")
    nch = SHARD // 128
    score = np.concatenate(
        [gres[c]["gout"][:, :nch].T.ravel() for c in range(NCORES)])
    idx = np.concatenate(
        [gres[c]["gout"][:, nch:].T.ravel() for c in range(NCORES)]
    ).astype(np.int64)

    def _prep_w1(w):          # [C, HH] -> [128, sum(blocks)*KC*128]
        kc, mch = C // 128, HH // 128
        wr4 = w.reshape(kc, 128, mch, 128)
        parts = []
        m0 = 0
        for blen in W1_BLOCKS:
            blk = wr4[:, :, m0:m0 + blen, :]         # [kc,128,blen,128]
            parts.append(blk.transpose(1, 0, 2, 3).reshape(128, kc * blen * 128))
            m0 += blen
        return np.ascontiguousarray(np.concatenate(parts, axis=1))

    def _prep_w2(w):          # [HH, C] -> [KC*2, 128, KHALF*128]
        kc, khalf = C // 128, (HH // 128) // 2
        return np.ascontiguousarray(
            w.reshape(2, khalf, 128, kc, 128).transpose(3, 0, 2, 1, 4)
             .reshape(2 * kc, 128, khalf * 128))

    # ---- host all-to-all dispatch: pair experts, split H across 2 cores ----
    counts = np.bincount(idx, minlength=E)
    order = np.argsort(-counts, kind="stable")
    pairs = [(int(order[i]), int(order[E - 1 - i])) for i in range(E // 2)]
    feasible = all(counts[ea] <= CAPA and counts[eb] <= B0
                   for ea, eb in pairs)
    if not feasible:
        # Safety net for out-of-distribution routing (cannot happen for the
        # fixed problem seed): exact dense-per-token fallback on host.
        out = np.empty((S, C), dtype=np.float32)
        for e in range(E):
            tok = np.nonzero(idx == e)[0]
            if tok.size == 0:
                continue
            hmid = xf[tok] @ w1[e] + b1[e]
            from scipy.special import erf
            hmid = 0.5 * hmid * (1.0 + erf(hmid / np.sqrt(2.0)))
            out[tok] = ((hmid @ w2[e] + b2[e])
                        * score[tok][:, None]).astype(np.float32)
        return out.reshape(B, T, C), np.float32(out.sum(dtype=np.float32))

    token_lists = []   # per pair: (tok_a, tok_b)
    ffn_maps = []
    for p, (ea, eb) in enumerate(pairs):
        tok_a = np.nonzero(idx == ea)[0]
        tok_b = np.nonzero(idx == eb)[0]
        token_lists.append((tok_a, tok_b))
        xgt = np.zeros((C, TOT), dtype=np.float32)
        xgt[:, :tok_a.size] = xT[:, tok_a]
        xgt[:, CAPA:CAPA + tok_b.size] = xT[:, tok_b]
        sc = np.zeros(TOT, dtype=np.float32)
        sc[:tok_a.size] = score[tok_a]
        sc[CAPA:CAPA + tok_b.size] = score[tok_b]
        b2a = (b2[ea] * 0.5).astype(np.float32)
        b2b = (b2[eb] * 0.5).astype(np.float32)
        for half in range(2):
            hs = slice(half * HH, (half + 1) * HH)
            ffn_maps.append({
                "xgt": xgt, "sc": sc,
                "w1a": _prep_w1(w1[ea][:, hs]),
                "w1b": _prep_w1(w1[eb][:, hs]),
                "w2a": _prep_w2(w2[ea][hs, :]),
                "w2b": _prep_w2(w2[eb][hs, :]),
                "b1a": np.ascontiguousarray(b1[ea][hs]),
                "b1b": np.ascontiguousarray(b1[eb][hs]),
                "b2a": b2a, "b2b": b2b,
            })

    # ---- launch 2: expert FFN ----
    ffn_nc = _get("ffn", _build_ffn)
    fres = _run(ffn_nc, ffn_maps, "ffn")

    # ---- combine: host sums the two H-half partials, scatters by token ----
    out = np.empty((S, C), dtype=np.float32)
    total = np.float32(0.0)
    for p, (ea, eb) in enumerate(pairs):
        tok_a, tok_b = token_lists[p]
        o0 = fres[2 * p]["out"]
        o1 = fres[2 * p + 1]["out"]
        out[tok_a] = (o0[:, :tok_a.size] + o1[:, :tok_a.size]).T
        out[tok_b] = (o0[:, CAPA:CAPA + tok_b.size]
                      + o1[:, CAPA:CAPA + tok_b.size]).T
        total = np.float32(total + fres[2 * p]["tot"][0, 0]
                           + fres[2 * p + 1]["tot"][0, 0])
    return out.reshape(B, T, C), total
